# revision 1
# baseline (speedup 1.0000x reference)
"""Causal self-attention (B=4, S=2048, D=1024, H=16) on 8 TRN2 NeuronCores.

Sharding (tensor-parallel on heads + data-parallel on batch):
  core c -> batch c//2, head-half c%2 (8 of 16 heads).
  Wq/Wk/Wv column-split, Wo row-split; the two partial outputs per batch are
  summed on the host (+ bo), which is the row-parallel unshard.

Per-core Bass/Tile program (matmul operands bf16, psum/softmax fp32):
  phase A: qT/kT feature-major projections (4-moving-block stationary chains);
           v token-major with a per-head ones column, emitted per-superblock
           interleaved with attention to keep the PE stream dense.
  phase B: per head / 512-query superblock / 128-key tile:
           scoresT = k_j @ q_blk.T (keys on partitions, two heads on disjoint
           PE row groups), additive triangular mask on the diagonal boundary
           subtile, dead columns skipped in scores, exp and PV
           (no max subtraction: scores ~ N(0,1)); PV accumulation with the
           ones column producing sumexp in row 64; reciprocal broadcast via a
           K=1 matmul; PV emission software-pipelined one key tile behind
           scores to hide the exp latency.
  phase C: out_partial = attnT.T @ Wo_rows (stationary reused across the two
           output column blocks).
"""

from contextlib import ExitStack

import numpy as np
import ml_dtypes

import concourse.bass as bass
import concourse.bacc as bacc
import concourse.tile as tile
import concourse.mybir as mybir

F32 = mybir.dt.float32
F32R = mybir.dt.float32r
BF16 = mybir.dt.bfloat16
NEG = -30000.0  # additive mask; must stay finite-representable in bf16 paths


def r(ap):
    return ap.bitcast(F32R)


def build_core_program(S=2048, D=1024, HC=8, DH=64, SQ=512, mm_dt=BF16,
                       xt_bufs=2, qk_psum_bufs=4, probs_bufs=6):
    """Build the per-core Bass program (SPMD: same program, different data).
    mm_dt: dtype of matmul operands (BF16 or F32R). When BF16, the host must
    pass xT/wqk/wv/wo as bfloat16 arrays."""
    DQ = HC * DH              # head-slice width (512)
    DK = D // 128             # contraction tiles for projections (8)
    DQN = DQ // 128           # head-pair tiles (4)
    NSB = S // SQ             # query superblocks (4)
    NTT = S // 128            # token tiles (16)
    NOUT = min(512, D)        # output-proj free width
    NOB = D // NOUT           # output-proj col blocks (2)
    assert DQ % 128 == 0 and S % SQ == 0 and SQ % 128 == 0 and D % 128 == 0
    assert (S // SQ) % 2 == 0

    bf = mm_dt == BF16
    in_dt = BF16 if bf else F32

    def m(ap):
        # bitcast for f32->f32r reinterpretation; no-op for bf16 tiles
        return ap if bf else ap.bitcast(F32R)

    nc = bacc.Bacc("TRN2", target_bir_lowering=False, debug=False)

    xT = nc.dram_tensor("xT", [D, S], in_dt, kind="ExternalInput").ap()
    wqk = nc.dram_tensor("wqk", [D, 2 * DQ], in_dt, kind="ExternalInput").ap()
    wv = nc.dram_tensor("wv", [D, DQ], in_dt, kind="ExternalInput").ap()
    wo = nc.dram_tensor("wo", [DQ, D], in_dt, kind="ExternalInput").ap()
    bqk = nc.dram_tensor("bqk", [2 * DQ], F32, kind="ExternalInput").ap()
    bv = nc.dram_tensor("bv", [DQ], F32, kind="ExternalInput").ap()
    out = nc.dram_tensor("out", [S, D], F32, kind="ExternalOutput").ap()

    with tile.TileContext(nc) as tc, ExitStack() as ctx:
        ctx.enter_context(nc.allow_low_precision(
            reason="low-precision matmul operands; accumulation stays fp32"))
        const = ctx.enter_context(tc.tile_pool(name="const", bufs=1))
        big = ctx.enter_context(tc.tile_pool(name="big", bufs=1))
        stream = ctx.enter_context(tc.tile_pool(name="stream", bufs=1))
        psum = ctx.enter_context(tc.tile_pool(name="psum", bufs=1, space="PSUM"))

        # ---- constants ----
        # triangular mask [128,128]: 0 where p <= f else NEG (boundary subtile)
        tri = const.tile([128, 128], F32)
        nc.vector.memset(tri[:], 0.0)
        nc.gpsimd.affine_select(
            out=tri[:], in_=tri[:], compare_op=mybir.AluOpType.is_ge,
            fill=NEG, base=0, channel_multiplier=-1, pattern=[[1, 128]],
        )
        ones128f = const.tile([1, 128], F32)
        nc.vector.memset(ones128f[:], 1.0)
        ones64r = const.tile([1, 64], F32R)
        nc.vector.tensor_copy(ones64r[:], ones128f[:, 0:64])
        ones128r = const.tile([1, 128], F32R)
        nc.vector.tensor_copy(ones128r[:], ones128f[:])
        ones_hc = const.tile([128, HC], F32)
        nc.vector.memset(ones_hc[:], 1.0)

        # biases: bqk as [128, 2*DQN] (column t = dout tile t), bv broadcast
        bqk_sb = const.tile([128, 2 * DQN], F32)
        nc.sync.dma_start(bqk_sb[:], bqk.rearrange("(t p) -> p t", p=128))
        bv_rowf = const.tile([1, DQ], F32)
        nc.sync.dma_start(bv_rowf[:], bv.rearrange("(a d) -> a d", a=1))
        bv_row = const.tile([1, DQ], F32R)
        nc.vector.tensor_copy(bv_row[:], bv_rowf[:])
        bv_bc = const.tile([128, DQ], F32)
        bv_ps = psum.tile([128, DQ], F32, tag="v", bufs=2)
        nc.tensor.matmul(bv_ps[:], r(ones128r[:]), r(bv_row[:]),
                         start=True, stop=True)
        nc.scalar.copy(bv_bc[:], bv_ps[:])

        # ---- big resident tensors ----
        kT = big.tile([128, DQN, S], mm_dt)     # [pair 2x64 rows, tokens]
        qT = big.tile([128, DQN, S], mm_dt)
        v_aug = big.tile([128, NTT, HC * 65], mm_dt)
        wv_sb = big.tile([128, DK, DQ], mm_dt)
        wo_sb = big.tile([128, DQN, D], mm_dt)
        xt_all = big.tile([128, DK, S], mm_dt)

        for kt in range(DK):
            nc.sync.dma_start(xt_all[:, kt, :], m(xT[128 * kt:128 * (kt + 1), :]))

        # ===== phase A-qk: all projections, 4-moving-block stationary chains
        for dt in range(2 * DQN):
            wdt = stream.tile([128, DK, 128], mm_dt, tag="wdt", bufs=3)
            for kt in range(DK):
                nc.gpsimd.dma_start(
                    wdt[:, kt, :],
                    m(wqk[128 * kt:128 * (kt + 1), 128 * dt:128 * (dt + 1)]))
            pss = [psum.tile([128, SQ], F32, tag="qk", bufs=qk_psum_bufs,
                             name=f"pss_{dt}_{tb}") for tb in range(NSB)]
            for kt in range(DK):
                for tb in range(NSB):
                    nc.tensor.matmul(
                        pss[tb][:], m(wdt[:, kt, :]),
                        m(xt_all[:, kt, tb * SQ:(tb + 1) * SQ]),
                        start=(kt == 0), stop=(kt == DK - 1))
            is_q = dt < DQN
            hp = dt % DQN
            dest = qT if is_q else kT
            for tb in range(NSB):
                nc.scalar.activation(
                    dest[:, hp, tb * SQ:(tb + 1) * SQ], pss[tb][:],
                    mybir.ActivationFunctionType.Identity,
                    bias=bqk_sb[:, dt:dt + 1],
                    scale=0.125 if is_q else 1.0)

        def emit_v_group(blk):
            # v projection for token tiles of one superblock (token-stationary)
            for tt in range(blk * (SQ // 128), (blk + 1) * (SQ // 128)):
                psv = psum.tile([128, DQ], F32, tag="v", bufs=2,
                                name=f"psv_{tt}")
                for kt in range(DK):
                    nc.tensor.matmul(
                        psv[:], m(xt_all[:, kt, 128 * tt:128 * (tt + 1)]),
                        m(wv_sb[:, kt, :]),
                        start=(kt == 0), stop=(kt == DK - 1))
                va = v_aug[:, tt, :].rearrange("p (h c) -> p h c", h=HC)
                nc.vector.tensor_tensor(
                    va[:, :, 0:64], psv[:].rearrange("p (h c) -> p h c", h=HC),
                    bv_bc[:].rearrange("p (h c) -> p h c", h=HC),
                    op=mybir.AluOpType.add)
                nc.vector.tensor_copy(va[:, :, 64:65], ones_hc[:, :, None])

        for kt in range(DK):
            nc.gpsimd.dma_start(wv_sb[:, kt, :],
                                m(wv[128 * kt:128 * (kt + 1), :]))
        emit_v_group(0)
        for p4 in range(DQN):
            nc.gpsimd.dma_start(wo_sb[:, p4, :],
                                m(wo[128 * p4:128 * (p4 + 1), :]))

        for i in range(NSB):
            # ===== phase B: attention for superblock i =====================
            ND = SQ // 128
            NJ = ND * (i + 1)
            attnT = stream.tile([128, DQN, SQ], mm_dt, tag="attnT", bufs=2,
                                name=f"at_{i}")
            pending = [None]  # deferred (bc matmul + normalize) of prev hp
            for hp in range(DQN):
                pva = psum.tile([65, SQ], F32, tag="v", bufs=2,
                                name=f"pv_{i}_{hp}_0")
                pvb = psum.tile([65, SQ], F32, tag="v", bufs=2,
                                name=f"pv_{i}_{hp}_1")
                pvs = (pva, pvb)
                pend = None
                for j in range(NJ):
                    jj = j - ND * i
                    f0 = max(0, 128 * jj)
                    scs, prbs = [], []
                    for hh in range(2):
                        p0, p1 = 64 * hh, 64 * hh + 64
                        sc = psum.tile([128, SQ], F32, tag="qk",
                                       bufs=qk_psum_bufs,
                                       name=f"sc_{i}_{hp}_{j}_{hh}")
                        nc.tensor.matmul(
                            sc[:, f0:],
                            m(kT[p0:p1, hp, 128 * j:128 * (j + 1)]),
                            m(qT[p0:p1, hp, i * SQ + f0:(i + 1) * SQ]),
                            start=True, stop=True,
                            tile_position=(64 * hh, 0))
                        scs.append(sc)
                    if j == 1 and pending[0] is not None:
                        pending[0]()
                        pending[0] = None
                    for hh in range(2):
                        if jj >= 0:
                            nc.vector.tensor_tensor(
                                scs[hh][:, f0:f0 + 128],
                                scs[hh][:, f0:f0 + 128], tri[:],
                                op=mybir.AluOpType.add)
                        probs = stream.tile([128, SQ], mm_dt, tag="probs",
                                            bufs=probs_bufs,
                                            name=f"pr_{i}_{hp}_{j}_{hh}")
                        nc.scalar.activation(
                            probs[:, f0:], scs[hh][:, f0:],
                            mybir.ActivationFunctionType.Exp)
                        prbs.append(probs)
                    if pend is not None:
                        pprbs, pf0, pj = pend
                        for hh in range(2):
                            h = 2 * hp + hh
                            nc.tensor.matmul(
                                pvs[hh][:, pf0:],
                                m(v_aug[:, pj, 65 * h:65 * h + 65]),
                                m(pprbs[hh][:, pf0:]),
                                start=(pj == 0), stop=(pj == NJ - 1))
                    pend = (prbs, f0, j)
                pprbs, pf0, pj = pend
                for hh in range(2):
                    h = 2 * hp + hh
                    nc.tensor.matmul(
                        pvs[hh][:, pf0:],
                        m(v_aug[:, pj, 65 * h:65 * h + 65]),
                        m(pprbs[hh][:, pf0:]),
                        start=(pj == 0), stop=(pj == NJ - 1))
                recips = []
                for hh in range(2):
                    recip = stream.tile([1, SQ], F32R, tag="recip", bufs=4,
                                        name=f"rc_{i}_{hp}_{hh}")
                    nc.vector.reciprocal(recip[:], pvs[hh][64:65, :])
                    recips.append(recip)

                def make_norm(pvs=pvs, recips=recips, hp=hp, at=attnT, ii=i):
                    def emit():
                        for hh in range(2):
                            bc = psum.tile([64, SQ], F32, tag="out", bufs=2,
                                           name=f"bc_{ii}_{hp}_{hh}")
                            nc.tensor.matmul(bc[:], r(ones64r[:]),
                                             r(recips[hh][:]),
                                             start=True, stop=True)
                            bc_sb = stream.tile([64, SQ], F32, tag="bcs",
                                                bufs=2,
                                                name=f"bs_{ii}_{hp}_{hh}")
                            nc.vector.tensor_copy(bc_sb[:], bc[:])
                            if hh == 0:
                                nc.vector.tensor_tensor(
                                    at[0:64, hp, :],
                                    pvs[hh][0:64, :], bc_sb[:],
                                    op=mybir.AluOpType.mult)
                            else:
                                stage = stream.tile([64, SQ], mm_dt,
                                                    tag="stage", bufs=2,
                                                    name=f"st_{ii}_{hp}_{hh}")
                                nc.vector.tensor_tensor(
                                    stage[:], pvs[hh][0:64, :], bc_sb[:],
                                    op=mybir.AluOpType.mult)
                                nc.sync.dma_start(at[64:128, hp, :], stage[:])
                    return emit

                pending[0] = make_norm()

            if pending[0] is not None:
                pending[0]()
                pending[0] = None
            if i + 1 < NSB:
                emit_v_group(i + 1)

            # ===== phase C: output projection for superblock i ============
            for mm_ in range(SQ // 128):
                tt = i * (SQ // 128) + mm_
                pos = [psum.tile([128, NOUT], F32, tag="out", bufs=2,
                                 name=f"po_{tt}_{nb}") for nb in range(NOB)]
                for p4 in range(DQN):
                    for nb in range(NOB):
                        nc.tensor.matmul(
                            pos[nb][:],
                            m(attnT[:, p4, 128 * mm_:128 * (mm_ + 1)]),
                            m(wo_sb[:, p4, nb * NOUT:(nb + 1) * NOUT]),
                            start=(p4 == 0), stop=(p4 == DQN - 1))
                for nb in range(NOB):
                    osb = stream.tile([128, NOUT], F32, tag="osb", bufs=3,
                                      name=f"ob_{tt}_{nb}")
                    nc.vector.tensor_copy(osb[:], pos[nb][:])
                    nc.sync.dma_start(
                        out[128 * tt:128 * (tt + 1),
                            nb * NOUT:(nb + 1) * NOUT], osb[:])

    nc.compile()
    return nc

B, S, D, H = 4, 2048, 1024, 16
N_CORES = 8

_CACHED = {}


def _make_core_inputs(x, Wq, bq, Wk, bk, Wv, bv, Wo):
    DQ = D // 2

    def cast(a):
        return np.ascontiguousarray(a).astype(ml_dtypes.bfloat16)

    xTs = [cast(x[b].T) for b in range(B)]
    in_maps = []
    for c in range(N_CORES):
        b, hf = c // 2, c % 2
        sl = slice(hf * DQ, (hf + 1) * DQ)
        in_maps.append({
            "xT": xTs[b],
            "wqk": cast(np.concatenate([Wq[:, sl], Wk[:, sl]], axis=1)),
            "wv": cast(Wv[:, sl]),
            "wo": cast(Wo[sl, :]),
            "bqk": np.ascontiguousarray(
                np.concatenate([0.125 * bq[sl], bk[sl]])).astype(np.float32),
            "bv": np.ascontiguousarray(bv[sl]).astype(np.float32),
        })
    return in_maps


def kernel(x, Wq, bq, Wk, bk, Wv, bv, Wo, bo):
    import tempfile
    from concourse import bass_utils

    x = np.asarray(x, dtype=np.float32)
    Wq = np.asarray(Wq, dtype=np.float32)
    bq = np.asarray(bq, dtype=np.float32)
    Wk = np.asarray(Wk, dtype=np.float32)
    bk = np.asarray(bk, dtype=np.float32)
    Wv = np.asarray(Wv, dtype=np.float32)
    bv = np.asarray(bv, dtype=np.float32)
    Wo = np.asarray(Wo, dtype=np.float32)
    bo = np.asarray(bo, dtype=np.float32)

    if "nc" not in _CACHED:
        _CACHED["nc"] = build_core_program(S=S, D=D, HC=H // 2)
    nc = _CACHED["nc"]

    in_maps = _make_core_inputs(x, Wq, bq, Wk, bk, Wv, bv, Wo)
    res = bass_utils.run_bass_kernel_spmd(
        nc, in_maps, core_ids=list(range(N_CORES)),
        tmpdir=tempfile.mkdtemp(prefix="bass_attn_"))

    out = np.empty((B, S, D), dtype=np.float32)
    for b in range(B):
        out[b] = res.results[2 * b]["out"] + res.results[2 * b + 1]["out"] + bo
    return out



# revision 3
# speedup vs baseline: 1.3629x; 1.3629x over previous
"""Causal self-attention (B=4, S=2048, D=1024, H=16) on 8 TRN2 NeuronCores.

Sharding (tensor-parallel on heads + data-parallel on batch):
  core c -> batch c//2, head-half c%2 (8 of 16 heads).
  Wq/Wk/Wv column-split, Wo row-split; the two partial outputs per batch are
  summed on the host (+ bo), which is the row-parallel unshard.

Per-core Bass/Tile program (matmul operands bf16, psum/softmax fp32).
v2 changes vs the 462us baseline (trace-driven):
  - softmax normalization: reciprocal_approx_fast on the sumexp row (~5x
    cheaper than full-precision reciprocal) + gpsimd partition_broadcast
    (replaces the PE broadcast-matmul + DVE psum copy). Removes ~130us of
    DVE time and the per-head-pair critical-path stall.
  - exp fused across the two heads of a pair: scores land in one 2-bank
    [128,2,SQ] psum tile, a single ACTIVATE covers both (halves ACT
    instruction count; each ACTIVATE pays a ~350-cycle pipeline fill).
  - causal mask applied multiplicatively on bf16 probs (packed 2-byte DVE
    mode) instead of fp32 adds on psum scores.
  - projections pipelined by superblock pair so attention exp work starts
    ~60us earlier and the PE stream stays dense (HAM stays at 2.4 GHz).
  - wqk kept SBUF-resident; q scale 1/8 folded into Wq on host.
"""

from contextlib import ExitStack

import numpy as np
import ml_dtypes

import concourse.bass as bass
import concourse.bacc as bacc
import concourse.tile as tile
import concourse.mybir as mybir

F32 = mybir.dt.float32
F32R = mybir.dt.float32r
BF16 = mybir.dt.bfloat16


def build_core_program(S=2048, D=1024, HC=8, DH=64, SQ=512, mm_dt=BF16,
                       probs_bufs=6):
    """Build the per-core Bass program (SPMD: same program, different data).
    The host passes xT/wqk/wv/wo as bfloat16 with the 1/8 q-scale folded
    into the Wq columns of wqk."""
    DQ = HC * DH              # head-slice width (512)
    DK = D // 128             # contraction tiles for projections (8)
    DQN = DQ // 128           # head-pair tiles (4)
    NSB = S // SQ             # query superblocks (4)
    NTT = S // 128            # token tiles (16)
    NOUT = min(512, D)        # output-proj free width
    NOB = D // NOUT           # output-proj col blocks (2)
    ND = SQ // 128            # key tiles per superblock (4)
    assert DQ % 128 == 0 and S % SQ == 0 and SQ % 128 == 0 and D % 128 == 0

    nc = bacc.Bacc("TRN2", target_bir_lowering=False, debug=False)

    xT = nc.dram_tensor("xT", [D, S], mm_dt, kind="ExternalInput").ap()
    wqk = nc.dram_tensor("wqk", [D, 2 * DQ], mm_dt, kind="ExternalInput").ap()
    wv = nc.dram_tensor("wv", [D, DQ], mm_dt, kind="ExternalInput").ap()
    wo = nc.dram_tensor("wo", [DQ, D], mm_dt, kind="ExternalInput").ap()
    bqk = nc.dram_tensor("bqk", [2 * DQ], F32, kind="ExternalInput").ap()
    bv = nc.dram_tensor("bv", [DQ], F32, kind="ExternalInput").ap()
    out = nc.dram_tensor("out", [S, D], F32, kind="ExternalOutput").ap()

    with tile.TileContext(nc) as tc, ExitStack() as ctx:
        ctx.enter_context(nc.allow_low_precision(
            reason="low-precision matmul operands; accumulation stays fp32"))
        const = ctx.enter_context(tc.tile_pool(name="const", bufs=1))
        big = ctx.enter_context(tc.tile_pool(name="big", bufs=1))
        stream = ctx.enter_context(tc.tile_pool(name="stream", bufs=1))
        psum = ctx.enter_context(tc.tile_pool(name="psum", bufs=1, space="PSUM"))

        # ---- constants ----
        # lower-triangular (incl diagonal) 0/1 mask in bf16 for probs masking
        trif = const.tile([128, 128], F32)
        nc.vector.memset(trif[:], 1.0)
        nc.gpsimd.affine_select(
            out=trif[:], in_=trif[:], compare_op=mybir.AluOpType.is_ge,
            fill=0.0, base=0, channel_multiplier=-1, pattern=[[1, 128]],
        )
        tri01 = const.tile([128, 128], mm_dt)
        nc.vector.tensor_copy(tri01[:], trif[:])

        # biases: bqk as [128, 2*DQN] (column t = dout tile t), bv broadcast
        bqk_sb = const.tile([128, 2 * DQN], F32)
        nc.sync.dma_start(bqk_sb[:], bqk.rearrange("(t p) -> p t", p=128))
        bv_rowf = const.tile([1, DQ], F32)
        nc.sync.dma_start(bv_rowf[:], bv.rearrange("(a d) -> a d", a=1))
        bv_bc = const.tile([128, DQ], F32)
        nc.gpsimd.partition_broadcast(bv_bc[:], bv_rowf[:])

        # ---- big resident tensors ----
        kT = big.tile([128, DQN, S], mm_dt)     # [pair 2x64 rows, tokens]
        qT = big.tile([128, DQN, S], mm_dt)
        v_aug = big.tile([128, NTT, HC * 65], mm_dt)
        wqk_sb = big.tile([128, DK, 2 * DQ], mm_dt)
        wv_sb = big.tile([128, DK, DQ], mm_dt)
        wo_sb = big.tile([128, DQN, D], mm_dt)
        xt_all = big.tile([128, DK, S], mm_dt)

        for kt in range(DK):
            nc.sync.dma_start(xt_all[:, kt, :], xT[128 * kt:128 * (kt + 1), :])
            nc.gpsimd.dma_start(wqk_sb[:, kt, :], wqk[128 * kt:128 * (kt + 1), :])
        for kt in range(DK):
            nc.gpsimd.dma_start(wv_sb[:, kt, :], wv[128 * kt:128 * (kt + 1), :])
        for p4 in range(DQN):
            nc.gpsimd.dma_start(wo_sb[:, p4, :], wo[128 * p4:128 * (p4 + 1), :])

        # ones column of v_aug (column 64 of each head slot), set once
        va4 = v_aug[:].rearrange("p t (h c) -> p t h c", h=HC)
        nc.vector.memset(va4[:, :, :, 64:65], 1.0)

        def proj_qk(tbp):
            """q/k projections for superblock pair tbp (tokens tbp*2*SQ ...).
            One [128,2,SQ] psum pair per dout tile; stationary wqk slice
            reused across the two moving halves."""
            for dt in range(2 * DQN):
                pss = psum.tile([128, 2, SQ], F32, tag="sc", bufs=2,
                                name=f"pss_{tbp}_{dt}")
                for kt in range(DK):
                    for h in range(2):
                        tb = 2 * tbp + h
                        nc.tensor.matmul(
                            pss[:, h, :],
                            wqk_sb[:, kt, 128 * dt:128 * (dt + 1)],
                            xt_all[:, kt, tb * SQ:(tb + 1) * SQ],
                            start=(kt == 0), stop=(kt == DK - 1))
                is_q = dt < DQN
                hp = dt % DQN
                dest = qT if is_q else kT
                nc.scalar.activation(
                    dest[:, hp, 2 * tbp * SQ:2 * (tbp + 1) * SQ],
                    pss[:, :, :],
                    mybir.ActivationFunctionType.Identity,
                    bias=bqk_sb[:, dt:dt + 1])

        def proj_v(tt0, tt1):
            # v projection, token-stationary, bias added on DVE
            for tt in range(tt0, tt1):
                psv = psum.tile([128, DQ], F32, tag="out", bufs=2,
                                name=f"psv_{tt}")
                for kt in range(DK):
                    nc.tensor.matmul(
                        psv[:], xt_all[:, kt, 128 * tt:128 * (tt + 1)],
                        wv_sb[:, kt, :],
                        start=(kt == 0), stop=(kt == DK - 1))
                va = v_aug[:, tt, :].rearrange("p (h c) -> p h c", h=HC)
                nc.vector.tensor_tensor(
                    va[:, :, 0:64], psv[:].rearrange("p (h c) -> p h c", h=HC),
                    bv_bc[:].rearrange("p (h c) -> p h c", h=HC),
                    op=mybir.AluOpType.add)

        def attention(i, attnT):
            """scores/softmax/PV for query superblock i -> attnT (bf16)."""
            NJ = ND * (i + 1)
            for hp in range(DQN):
                pva = psum.tile([65, SQ], F32, tag="pv", bufs=2,
                                name=f"pv_{i}_{hp}_0")
                pvb = psum.tile([65, SQ], F32, tag="pv", bufs=2,
                                name=f"pv_{i}_{hp}_1")
                pvs = (pva, pvb)
                pend = None
                for j in range(NJ):
                    jj = j - ND * i
                    f0 = max(0, 128 * jj)
                    sc = psum.tile([128, 2, SQ], F32, tag="sc", bufs=2,
                                   name=f"sc_{i}_{hp}_{j}")
                    for hh in range(2):
                        p0, p1 = 64 * hh, 64 * hh + 64
                        nc.tensor.matmul(
                            sc[:, hh, f0:],
                            kT[p0:p1, hp, 128 * j:128 * (j + 1)],
                            qT[p0:p1, hp, i * SQ + f0:(i + 1) * SQ],
                            start=True, stop=True,
                            tile_position=(64 * hh, 0))
                    probs = stream.tile([128, 2, SQ], mm_dt, tag="probs",
                                        bufs=probs_bufs,
                                        name=f"pr_{i}_{hp}_{j}")
                    nc.scalar.activation(
                        probs[:, :, f0:], sc[:, :, f0:],
                        mybir.ActivationFunctionType.Exp)
                    if jj >= 0:
                        # diagonal boundary tile: zero the upper triangle
                        for hh in range(2):
                            nc.vector.tensor_tensor(
                                probs[:, hh, f0:f0 + 128],
                                probs[:, hh, f0:f0 + 128], tri01[:],
                                op=mybir.AluOpType.mult)
                    if pend is not None:
                        pprbs, pf0, pj = pend
                        for hh in range(2):
                            h = 2 * hp + hh
                            nc.tensor.matmul(
                                pvs[hh][:, pf0:],
                                v_aug[:, pj, 65 * h:65 * h + 65],
                                pprbs[:, hh, pf0:],
                                start=(pj == 0), stop=(pj == NJ - 1))
                    pend = (probs, f0, j)
                pprbs, pf0, pj = pend
                for hh in range(2):
                    h = 2 * hp + hh
                    nc.tensor.matmul(
                        pvs[hh][:, pf0:],
                        v_aug[:, pj, 65 * h:65 * h + 65],
                        pprbs[:, hh, pf0:],
                        start=(pj == 0), stop=(pj == NJ - 1))
                # normalization: recip of sumexp row (DVE, fast approx),
                # broadcast to 64 partitions (gpsimd), scale (DVE)
                for hh in range(2):
                    # custom-DVE ops misread partition-shifted rows; stage the
                    # sumexp row at partition 0 with a plain copy first
                    srow = stream.tile([1, SQ], F32, tag="srow", bufs=4,
                                       name=f"sr_{i}_{hp}_{hh}")
                    nc.vector.tensor_copy(srow[:], pvs[hh][64:65, :])
                    rc = stream.tile([1, SQ], F32, tag="rc", bufs=4,
                                     name=f"rc_{i}_{hp}_{hh}")
                    nc.vector.reciprocal_approx_fast(out=rc[:], in_=srow[:])
                    bc = stream.tile([64, SQ], F32, tag="bc", bufs=4,
                                     name=f"bc_{i}_{hp}_{hh}")
                    nc.gpsimd.partition_broadcast(bc[:], rc[:])
                    if hh == 0:
                        nc.vector.tensor_tensor(
                            attnT[0:64, hp, :], pvs[hh][0:64, :], bc[:],
                            op=mybir.AluOpType.mult)
                    else:
                        stage = stream.tile([64, SQ], mm_dt, tag="stage",
                                            bufs=2, name=f"st_{i}_{hp}")
                        nc.vector.tensor_tensor(
                            stage[:], pvs[hh][0:64, :], bc[:],
                            op=mybir.AluOpType.mult)
                        nc.sync.dma_start(attnT[64:128, hp, :], stage[:])

        def out_proj(i, attnT):
            for mm_ in range(SQ // 128):
                tt = i * (SQ // 128) + mm_
                pos = [psum.tile([128, NOUT], F32, tag="out", bufs=2,
                                 name=f"po_{tt}_{nb}") for nb in range(NOB)]
                for p4 in range(DQN):
                    for nb in range(NOB):
                        nc.tensor.matmul(
                            pos[nb][:],
                            attnT[:, p4, 128 * mm_:128 * (mm_ + 1)],
                            wo_sb[:, p4, nb * NOUT:(nb + 1) * NOUT],
                            start=(p4 == 0), stop=(p4 == DQN - 1))
                for nb in range(NOB):
                    osb = stream.tile([128, NOUT], F32, tag="osb", bufs=3,
                                      name=f"ob_{tt}_{nb}")
                    nc.vector.tensor_copy(osb[:], pos[nb][:])
                    nc.sync.dma_start(
                        out[128 * tt:128 * (tt + 1),
                            nb * NOUT:(nb + 1) * NOUT], osb[:])

        ats = [stream.tile([128, DQN, SQ], mm_dt, tag="attnT", bufs=2,
                           name=f"at_{i}") for i in range(NSB)]
        # pipelined schedule: attention(i) exp work overlaps the projection
        # and out-proj matmul stretches around it
        proj_qk(0)                    # q/k for superblocks 0,1
        proj_v(0, 2 * ND)             # v for superblocks 0,1
        attention(0, ats[0])
        proj_qk(1)                    # q/k for superblocks 2,3
        out_proj(0, ats[0])
        attention(1, ats[1])
        proj_v(2 * ND, 4 * ND)        # v for superblocks 2,3
        out_proj(1, ats[1])
        attention(2, ats[2])
        out_proj(2, ats[2])
        attention(3, ats[3])
        out_proj(3, ats[3])

    nc.compile()
    return nc

B, S, D, H = 4, 2048, 1024, 16
N_CORES = 8

_CACHED = {}


def _make_core_inputs(x, Wq, bq, Wk, bk, Wv, bv, Wo):
    DQ = D // 2

    def cast(a):
        return np.ascontiguousarray(a).astype(ml_dtypes.bfloat16)

    xTs = [cast(x[b].T) for b in range(B)]
    in_maps = []
    for c in range(N_CORES):
        b, hf = c // 2, c % 2
        sl = slice(hf * DQ, (hf + 1) * DQ)
        in_maps.append({
            "xT": xTs[b],
            "wqk": cast(np.concatenate([0.125 * Wq[:, sl], Wk[:, sl]],
                                       axis=1)),
            "wv": cast(Wv[:, sl]),
            "wo": cast(Wo[sl, :]),
            "bqk": np.ascontiguousarray(
                np.concatenate([0.125 * bq[sl], bk[sl]])).astype(np.float32),
            "bv": np.ascontiguousarray(bv[sl]).astype(np.float32),
        })
    return in_maps


def kernel(x, Wq, bq, Wk, bk, Wv, bv, Wo, bo):
    import tempfile
    from concourse import bass_utils

    x = np.asarray(x, dtype=np.float32)
    Wq = np.asarray(Wq, dtype=np.float32)
    bq = np.asarray(bq, dtype=np.float32)
    Wk = np.asarray(Wk, dtype=np.float32)
    bk = np.asarray(bk, dtype=np.float32)
    Wv = np.asarray(Wv, dtype=np.float32)
    bv = np.asarray(bv, dtype=np.float32)
    Wo = np.asarray(Wo, dtype=np.float32)
    bo = np.asarray(bo, dtype=np.float32)

    if "nc" not in _CACHED:
        _CACHED["nc"] = build_core_program(S=S, D=D, HC=H // 2)
    nc = _CACHED["nc"]

    in_maps = _make_core_inputs(x, Wq, bq, Wk, bk, Wv, bv, Wo)
    res = bass_utils.run_bass_kernel_spmd(
        nc, in_maps, core_ids=list(range(N_CORES)),
        tmpdir=tempfile.mkdtemp(prefix="bass_attn_"))

    out = np.empty((B, S, D), dtype=np.float32)
    for b in range(B):
        out[b] = res.results[2 * b]["out"] + res.results[2 * b + 1]["out"] + bo
    return out


# revision 7
# speedup vs baseline: 1.4786x; 1.0849x over previous
"""Causal self-attention (B=4, S=2048, D=1024, H=16) on 8 TRN2 NeuronCores.

Sharding (tensor-parallel on heads + data-parallel on batch):
  core c -> batch c//2, head-half c%2 (8 of 16 heads).
  Wq/Wk/Wv column-split, Wo row-split; the two partial outputs per batch are
  summed on the host (+ bo), which is the row-parallel unshard.

Per-core Bass/Tile program (matmul operands bf16, psum/softmax fp32).
v4 design (trace-driven):
  - Attention phase is ACT(exp)-bound per key tile (one fused [128,2,SQ]
    ACTIVATE per tile vs PE ~0.85us), so projection / out-projection matmul
    chains are interleaved as "fillers" inside the attention loops: the PE
    FIFO always has exp-independent work while ACT grinds exps.
  - softmax normalization: partition-shifted copy of the sumexp row +
    reciprocal_approx_fast + gpsimd partition_broadcast (no PE broadcast
    matmul, no full-precision reciprocal).
  - causal masking: multiplicative triangular zeroing of bf16 probs on the
    otherwise-idle gpsimd engine (affine_select), off the DVE.
  - phase-A psum->sbuf copies with bias on DVE (tensor_scalar per-partition
    add), keeping the ACT queue pure exps during attention.
  - q/k/v projections split per superblock pair and scheduled so exp work
    starts as early as possible; input DMAs spread over idle engine queues.
"""

from contextlib import ExitStack

import numpy as np
import ml_dtypes

import concourse.bass as bass
import concourse.bacc as bacc
import concourse.tile as tile
import concourse.mybir as mybir

F32 = mybir.dt.float32
BF16 = mybir.dt.bfloat16


def build_core_program(S=2048, D=1024, HC=8, DH=64, SQ=512, mm_dt=BF16,
                       probs_bufs=6):
    """Build the per-core Bass program (SPMD: same program, different data).
    The host passes xT/wqk/wv/wo as bfloat16 with the 1/8 q-scale folded
    into the Wq columns of wqk."""
    DQ = HC * DH              # head-slice width (512)
    DK = D // 128             # contraction tiles for projections (8)
    DQN = DQ // 128           # head-pair tiles (4)
    NSB = S // SQ             # query superblocks (4)
    NTT = S // 128            # token tiles (16)
    NOUT = min(512, D)        # output-proj free width
    NOB = D // NOUT           # output-proj col blocks (2)
    ND = SQ // 128            # key tiles per superblock (4)
    assert DQ % 128 == 0 and S % SQ == 0 and SQ % 128 == 0 and D % 128 == 0

    nc = bacc.Bacc("TRN2", target_bir_lowering=False, debug=False)

    xT = nc.dram_tensor("xT", [D, S], mm_dt, kind="ExternalInput").ap()
    wqk = nc.dram_tensor("wqk", [D, 2 * DQ], mm_dt, kind="ExternalInput").ap()
    wv = nc.dram_tensor("wv", [D, DQ], mm_dt, kind="ExternalInput").ap()
    wo = nc.dram_tensor("wo", [DQ, D], mm_dt, kind="ExternalInput").ap()
    bqk = nc.dram_tensor("bqk", [2 * DQ], F32, kind="ExternalInput").ap()
    bv = nc.dram_tensor("bv", [DQ], F32, kind="ExternalInput").ap()
    out = nc.dram_tensor("out", [S, D], F32, kind="ExternalOutput").ap()

    with tile.TileContext(nc) as tc, ExitStack() as ctx:
        ctx.enter_context(nc.allow_low_precision(
            reason="low-precision matmul operands; accumulation stays fp32"))
        const = ctx.enter_context(tc.tile_pool(name="const", bufs=1))
        big = ctx.enter_context(tc.tile_pool(name="big", bufs=1))
        stream = ctx.enter_context(tc.tile_pool(name="stream", bufs=1))
        psum = ctx.enter_context(tc.tile_pool(name="psum", bufs=1, space="PSUM"))

        # biases: bqk as [128, 2*DQN] (column t = dout tile t), bv broadcast
        bqk_sb = const.tile([128, 2 * DQN], F32)
        nc.sync.dma_start(bqk_sb[:], bqk.rearrange("(t p) -> p t", p=128))
        bv_rowf = const.tile([1, DQ], F32)
        nc.sync.dma_start(bv_rowf[:], bv.rearrange("(a d) -> a d", a=1))
        bv_bc = const.tile([128, DQ], F32)
        nc.gpsimd.partition_broadcast(bv_bc[:], bv_rowf[:])

        # ---- big resident tensors ----
        kT = big.tile([128, DQN, S], mm_dt)     # [pair 2x64 rows, tokens]
        qT = big.tile([128, DQN, S], mm_dt)
        v_aug = big.tile([128, NTT, HC * 65], mm_dt)
        wqk_sb = big.tile([128, DK, 2 * DQ], mm_dt)
        wv_sb = big.tile([128, DK, DQ], mm_dt)
        wo_sb = big.tile([128, DQN, D], mm_dt)
        xt_all = big.tile([128, DK, S], mm_dt)

        # x split over two idle engine DMA queues (only SP/ACT/gpsimd can
        # initiate DMAs); weights go on the gpsimd queue
        xq = [nc.sync, nc.scalar]
        for kt in range(DK):
            xq[kt % 2].dma_start(xt_all[:, kt, :],
                                 xT[128 * kt:128 * (kt + 1), :])
        for kt in range(DK):
            nc.gpsimd.dma_start(wqk_sb[:, kt, :], wqk[128 * kt:128 * (kt + 1), :])
        for kt in range(DK):
            nc.gpsimd.dma_start(wv_sb[:, kt, :], wv[128 * kt:128 * (kt + 1), :])
        for p4 in range(DQN):
            nc.gpsimd.dma_start(wo_sb[:, p4, :], wo[128 * p4:128 * (p4 + 1), :])

        # ones column of v_aug (column 64 of each head slot), set once
        va4 = v_aug[:].rearrange("p t (h c) -> p t h c", h=HC)
        nc.vector.memset(va4[:, :, :, 64:65], 1.0)

        def proj_qk_unit(tbp, dt):
            """q/k projection chain: one dout tile, superblock pair tbp."""
            pss = psum.tile([128, 2, SQ], F32, tag="sc", bufs=2,
                            name=f"pss_{tbp}_{dt}")
            for kt in range(DK):
                for h in range(2):
                    tb = 2 * tbp + h
                    nc.tensor.matmul(
                        pss[:, h, :],
                        wqk_sb[:, kt, 128 * dt:128 * (dt + 1)],
                        xt_all[:, kt, tb * SQ:(tb + 1) * SQ],
                        start=(kt == 0), stop=(kt == DK - 1))
            dest = qT if dt < DQN else kT
            hp = dt % DQN
            nc.vector.tensor_scalar(
                out=dest[:, hp, 2 * tbp * SQ:2 * (tbp + 1) * SQ],
                in0=pss[:].rearrange("p a b -> p (a b)"),
                scalar1=bqk_sb[:, dt:dt + 1], scalar2=None,
                op0=mybir.AluOpType.add)

        def proj_v_unit(tt):
            # v projection for one token tile, bias added on DVE
            psv = psum.tile([128, DQ], F32, tag="out", bufs=2,
                            name=f"psv_{tt}")
            for kt in range(DK):
                nc.tensor.matmul(
                    psv[:], xt_all[:, kt, 128 * tt:128 * (tt + 1)],
                    wv_sb[:, kt, :],
                    start=(kt == 0), stop=(kt == DK - 1))
            va = v_aug[:, tt, :].rearrange("p (h c) -> p h c", h=HC)
            nc.vector.tensor_tensor(
                va[:, :, 0:64], psv[:].rearrange("p (h c) -> p h c", h=HC),
                bv_bc[:].rearrange("p (h c) -> p h c", h=HC),
                op=mybir.AluOpType.add)

        def out_unit(i, attnT, mm_):
            # out-projection for one token tile of superblock i
            tt = i * ND + mm_
            pos = [psum.tile([128, NOUT], F32, tag="out", bufs=2,
                             name=f"po_{tt}_{nb}") for nb in range(NOB)]
            for p4 in range(DQN):
                for nb in range(NOB):
                    nc.tensor.matmul(
                        pos[nb][:],
                        attnT[:, p4, 128 * mm_:128 * (mm_ + 1)],
                        wo_sb[:, p4, nb * NOUT:(nb + 1) * NOUT],
                        start=(p4 == 0), stop=(p4 == DQN - 1))
            for nb in range(NOB):
                osb = stream.tile([128, NOUT], F32, tag="osb", bufs=3,
                                  name=f"ob_{tt}_{nb}")
                nc.vector.tensor_copy(osb[:], pos[nb][:])
                nc.sync.dma_start(
                    out[128 * tt:128 * (tt + 1),
                        nb * NOUT:(nb + 1) * NOUT], osb[:])

        def attention(i, attnT, fillers):
            """scores/softmax/PV for query superblock i -> attnT (bf16).
            fillers: exp-independent PE work drained inside the loops."""
            fillers = list(fillers)

            def drain():
                if fillers:
                    fillers.pop(0)()

            NJ = ND * (i + 1)
            dp = sorted({max(1, NJ // 3), max(2, (2 * NJ) // 3)})
            for hp in range(DQN):
                pva = psum.tile([65, SQ], F32, tag="pv", bufs=2,
                                name=f"pv_{i}_{hp}_0")
                pvb = psum.tile([65, SQ], F32, tag="pv", bufs=2,
                                name=f"pv_{i}_{hp}_1")
                pvs = (pva, pvb)
                pend = None
                for j in range(NJ):
                    jj = j - ND * i
                    f0 = max(0, 128 * jj)
                    sc = psum.tile([128, 2, SQ], F32, tag="sc", bufs=2,
                                   name=f"sc_{i}_{hp}_{j}")
                    for hh in range(2):
                        p0, p1 = 64 * hh, 64 * hh + 64
                        nc.tensor.matmul(
                            sc[:, hh, f0:],
                            kT[p0:p1, hp, 128 * j:128 * (j + 1)],
                            qT[p0:p1, hp, i * SQ + f0:(i + 1) * SQ],
                            start=True, stop=True,
                            tile_position=(64 * hh, 0))
                    probs = stream.tile([128, 2, SQ], mm_dt, tag="probs",
                                        bufs=probs_bufs,
                                        name=f"pr_{i}_{hp}_{j}")
                    nc.scalar.activation(
                        probs[:, :, f0:], sc[:, :, f0:],
                        mybir.ActivationFunctionType.Exp)
                    if jj >= 0:
                        # diagonal boundary tile: zero probs above the
                        # diagonal (gpsimd, keeps DVE off the exp->PV path)
                        for hh in range(2):
                            nc.gpsimd.affine_select(
                                out=probs[:, hh, f0:f0 + 128],
                                in_=probs[:, hh, f0:f0 + 128],
                                compare_op=mybir.AluOpType.is_ge,
                                fill=0.0, base=0, channel_multiplier=-1,
                                pattern=[[1, 128]])
                    if pend is not None:
                        pprbs, pf0, pj = pend
                        for hh in range(2):
                            h = 2 * hp + hh
                            nc.tensor.matmul(
                                pvs[hh][:, pf0:],
                                v_aug[:, pj, 65 * h:65 * h + 65],
                                pprbs[:, hh, pf0:],
                                start=(pj == 0), stop=(pj == NJ - 1))
                    pend = (probs, f0, j)
                    if j in dp:
                        drain()
                pprbs, pf0, pj = pend
                for hh in range(2):
                    h = 2 * hp + hh
                    nc.tensor.matmul(
                        pvs[hh][:, pf0:],
                        v_aug[:, pj, 65 * h:65 * h + 65],
                        pprbs[:, hh, pf0:],
                        start=(pj == 0), stop=(pj == NJ - 1))
                # normalization: shifted copy of the sumexp row, fast recip,
                # gpsimd broadcast, per-column scale
                for hh in range(2):
                    srow = stream.tile([1, SQ], F32, tag="srow", bufs=4,
                                       name=f"sr_{i}_{hp}_{hh}")
                    nc.vector.tensor_copy(srow[:], pvs[hh][64:65, :])
                    rc = stream.tile([1, SQ], F32, tag="rc", bufs=4,
                                     name=f"rc_{i}_{hp}_{hh}")
                    nc.vector.reciprocal_approx_fast(out=rc[:], in_=srow[:])
                    bc = stream.tile([64, SQ], F32, tag="bc", bufs=4,
                                     name=f"bc_{i}_{hp}_{hh}")
                    nc.gpsimd.partition_broadcast(bc[:], rc[:])
                    if hh == 0:
                        nc.vector.tensor_tensor(
                            attnT[0:64, hp, :], pvs[hh][0:64, :], bc[:],
                            op=mybir.AluOpType.mult)
                    else:
                        stage = stream.tile([64, SQ], mm_dt, tag="stage",
                                            bufs=2, name=f"st_{i}_{hp}")
                        nc.vector.tensor_tensor(
                            stage[:], pvs[hh][0:64, :], bc[:],
                            op=mybir.AluOpType.mult)
                        nc.sync.dma_start(attnT[64:128, hp, :], stage[:])
                drain()
            while fillers:
                fillers.pop(0)()

        # all four attnT tiles stay live: out-proj fillers for superblock i
        # run inside attention(i+2), so no slot reuse is safe
        ats = [stream.tile([128, DQN, SQ], mm_dt, tag="attnT", bufs=NSB,
                           name=f"at_{i}") for i in range(NSB)]
        mk = lambda f, *a: (lambda: f(*a))

        # q/k for superblock-pair 0, head-pair 0 + v tiles 0-3 upfront so
        # attention(0) head-pair 0 can start immediately; everything else
        # drains as fillers inside the ACT-bound attention loops.
        P = lambda: None  # padding: spreads fillers across drain points
        proj_qk_unit(0, 0)
        proj_qk_unit(0, DQN)
        for tt in range(ND):
            proj_v_unit(tt)
        attention(0, ats[0], [
            mk(proj_qk_unit, 0, 1), mk(proj_qk_unit, 0, DQN + 1),
            mk(proj_qk_unit, 0, 2), mk(proj_qk_unit, 0, DQN + 2),
            mk(proj_qk_unit, 0, 3), mk(proj_qk_unit, 0, DQN + 3),
            mk(proj_v_unit, ND), mk(proj_v_unit, ND + 1),
            mk(proj_v_unit, ND + 2), mk(proj_v_unit, ND + 3),
        ])
        attention(1, ats[1], [
            mk(proj_qk_unit, 1, 0), mk(proj_qk_unit, 1, DQN),
            mk(proj_qk_unit, 1, 1), mk(proj_qk_unit, 1, DQN + 1),
            mk(proj_qk_unit, 1, 2), mk(proj_qk_unit, 1, DQN + 2),
            mk(proj_qk_unit, 1, 3), mk(proj_qk_unit, 1, DQN + 3),
            mk(proj_v_unit, 2 * ND), mk(proj_v_unit, 2 * ND + 1),
            mk(proj_v_unit, 2 * ND + 2), mk(proj_v_unit, 2 * ND + 3),
        ])
        attention(2, ats[2], [
            mk(proj_v_unit, 3 * ND), mk(proj_v_unit, 3 * ND + 1),
            mk(proj_v_unit, 3 * ND + 2), mk(proj_v_unit, 3 * ND + 3),
            mk(out_unit, 0, ats[0], 0), mk(out_unit, 0, ats[0], 1),
            mk(out_unit, 0, ats[0], 2), mk(out_unit, 0, ats[0], 3),
            mk(out_unit, 1, ats[1], 0), mk(out_unit, 1, ats[1], 1),
        ])
        attention(3, ats[3], [
            mk(out_unit, 1, ats[1], 2), P, mk(out_unit, 1, ats[1], 3), P,
            mk(out_unit, 2, ats[2], 0), P, mk(out_unit, 2, ats[2], 1), P,
            mk(out_unit, 2, ats[2], 2), P, mk(out_unit, 2, ats[2], 3), P,
        ])
        for mm_ in range(ND):
            out_unit(3, ats[3], mm_)

    nc.compile()
    return nc

B, S, D, H = 4, 2048, 1024, 16
N_CORES = 8

_CACHED = {}


def _make_core_inputs(x, Wq, bq, Wk, bk, Wv, bv, Wo):
    DQ = D // 2

    def cast(a):
        return np.ascontiguousarray(a).astype(ml_dtypes.bfloat16)

    xTs = [cast(x[b].T) for b in range(B)]
    in_maps = []
    for c in range(N_CORES):
        b, hf = c // 2, c % 2
        sl = slice(hf * DQ, (hf + 1) * DQ)
        in_maps.append({
            "xT": xTs[b],
            "wqk": cast(np.concatenate([0.125 * Wq[:, sl], Wk[:, sl]],
                                       axis=1)),
            "wv": cast(Wv[:, sl]),
            "wo": cast(Wo[sl, :]),
            "bqk": np.ascontiguousarray(
                np.concatenate([0.125 * bq[sl], bk[sl]])).astype(np.float32),
            "bv": np.ascontiguousarray(bv[sl]).astype(np.float32),
        })
    return in_maps


def kernel(x, Wq, bq, Wk, bk, Wv, bv, Wo, bo):
    import tempfile
    from concourse import bass_utils

    x = np.asarray(x, dtype=np.float32)
    Wq = np.asarray(Wq, dtype=np.float32)
    bq = np.asarray(bq, dtype=np.float32)
    Wk = np.asarray(Wk, dtype=np.float32)
    bk = np.asarray(bk, dtype=np.float32)
    Wv = np.asarray(Wv, dtype=np.float32)
    bv = np.asarray(bv, dtype=np.float32)
    Wo = np.asarray(Wo, dtype=np.float32)
    bo = np.asarray(bo, dtype=np.float32)

    if "nc" not in _CACHED:
        _CACHED["nc"] = build_core_program(S=S, D=D, HC=H // 2)
    nc = _CACHED["nc"]

    in_maps = _make_core_inputs(x, Wq, bq, Wk, bk, Wv, bv, Wo)
    res = bass_utils.run_bass_kernel_spmd(
        nc, in_maps, core_ids=list(range(N_CORES)),
        tmpdir=tempfile.mkdtemp(prefix="bass_attn_"))

    out = np.empty((B, S, D), dtype=np.float32)
    for b in range(B):
        out[b] = res.results[2 * b]["out"] + res.results[2 * b + 1]["out"] + bo
    return out
